# revision 32
# baseline (speedup 1.0000x reference)
"""DGCNN (4x EdgeConv + FC head) Bass kernel for 8 trn2 NeuronCores — v3.

Sharding: cloud b -> cores {2b, 2b+1}; each core owns 1024 query points
(q0 = (pid % 2) * 1024). Full cloud features exchanged within each pair via
bf16 AllGather after layers 1-3.

v3 changes vs v2 baseline:
- u injected via 128-query selector matmuls (selB[qh]) straight from the
  PSUM-evicted u tile: kills the per-tile DRAM bounce (2 DMAs/tile).
- Single SWDGE queue, one 2560-edge gather per tile (994ns fixed descgen
  cost amortized 4x); index replication shrinks to blocks {0,1}.
- L1 dist/u matmuls fold nsq + bias rows into the 5-row contract (matmul
  cost is free-size only), halving L1 dist PE time.
"""
import numpy as np
import ml_dtypes

import concourse.bass as bass
import concourse.bacc as bacc
import concourse.mybir as mybir
import concourse.tile as tile
from concourse.bass_utils import run_bass_kernel_spmd

B, P, K = 4, 2048, 20
NQ = 1024
N_CORES = 8
NGA = 4            # gathers per tile
FP8H1 = (3, 4)     # layers whose h1 uses the fp8 table + DoubleRow
GBLOCKS = (1, 3, 5, 7)   # wrapped idx replication blocks
NEG = -3.0e38
F32 = mybir.dt.float32
FP8 = mybir.dt.float8e4
BF16 = mybir.dt.bfloat16
U32 = mybir.dt.uint32
U16 = mybir.dt.uint16
I16 = mybir.dt.int16
AF = mybir.ActivationFunctionType
ALU = mybir.AluOpType
AX = mybir.AxisListType

#          D  Dpad  DH   DO
LCFG = [(3, 128, 64, 64),
        (64, 128, 128, 128),
        (128, 128, 256, 256),
        (256, 256, 512, 512)]
FC1_CHUNKS = [64, 128, 128, 128, 128, 128, 128, 128]  # 960 rows

_CACHED = {}


def cdiv(a, b):
    return (a + b - 1) // b


def _wpack_layout():
    """(key, rows, cols, col_off) chunks of the packed bf16 weight tensor."""
    lay, off = [], 0

    def add(key, r, c):
        nonlocal off
        lay.append((key, r, c, off))
        off += c

    for li, (D, DP, DH, DO) in enumerate(LCFG, start=1):
        if li == 1:
            add("wdiff1", 5, DH)  # 3 feature rows + ba hi/lo
        else:
            for c0 in range(0, D, 128):
                add(f"wdiff{li}", min(128, D - c0), DH)
            add(f"ba{li}", 2, DH)
        if li not in FP8H1:
            for c0 in range(0, DP, 128):
                add(f"wbot{li}", 128, DH)
    add("ident", 128, 128)
    for qh in range(8):
        add("selB", 128, 320)
    for r in FC1_CHUNKS:
        add("fw1", r, 512)
    for _ in range(4):
        add("fw2", 128, 256)
    for _ in range(2):
        add("fw3", 128, 1)
    return lay, off


def _w8pack_layout():
    """(key, pk, npair, cols, col_off) chunks of the packed fp8 wb tensor.

    Each chunk holds one DoubleRow pair: [pk, 2, DO] -> 2*DO flat columns.
    Pairs beyond DH are zero-padded (their h1sb subtiles are zeroed too).
    """
    lay, off = [], 0
    for li, (D, DP, DH, DO) in enumerate(LCFG, start=1):
        pk = min(128, DH)
        npair = cdiv(max(2, cdiv(DH, 128)), 2)
        for pr in range(npair):
            lay.append((f"wb8_{li}", pk, pr, 2 * DO, off))
            off += 2 * DO
        lay.append((f"wbot8_{li}", 128, 0, 2 * DH, off))
        off += 2 * DH
    return lay, off


def _fpack_layout():
    """(key, rows, cols, col_off) chunks of the packed f32 bias tensor."""
    lay, off = [], 0

    def add(key, r, c):
        nonlocal off
        lay.append((key, r, c, off))
        off += c

    for li, (_, _, _, DO) in enumerate(LCFG, start=1):
        for c0 in range(0, DO, 128):
            add(f"bb{li}", min(128, DO - c0), 1)
    for c0 in range(0, 512, 128):
        add("fb1", 128, 1)
    for c0 in range(0, 256, 128):
        add("fb2", 128, 1)
    add("fb3", 1, 1)
    return lay, off


def _build():
    nc = bacc.Bacc("TRN2", target_bir_lowering=False, debug=False,
                   num_devices=N_CORES, num_swdge_queues=max(1, NGA),
                   dynamic_dma_scratch_size=16384)

    # ---------------- DRAM params ----------------
    # xin rows: 0-4 lhs (x^T own + two ones), 5-9 rhs-own (x^T own + nsq
    # hi/lo), 10-14 rhs-twin (x^T twin + nsq hi/lo)
    xin_in = nc.declare_dram_parameter("xin", [15, NQ], BF16, isOutput=False)
    xsb1_in = nc.declare_dram_parameter("xsb1", [128, 16 * 128], BF16, isOutput=False)
    wlay, wcols = _wpack_layout()
    w8lay, w8cols = _w8pack_layout()
    flay, fcols = _fpack_layout()
    wpack_in = nc.declare_dram_parameter("wpack", [128, wcols], BF16, isOutput=False)
    w8pack_in = nc.declare_dram_parameter("w8pack", [128, w8cols], FP8,
                                          isOutput=False)
    fpack_in = nc.declare_dram_parameter("fpack", [128, fcols], F32, isOutput=False)
    y_out = nc.declare_dram_parameter("y", [1, NQ], F32, isOutput=True)

    groups = [[2 * b, 2 * b + 1] for b in range(N_CORES // 2)]

    with tile.TileContext(nc) as tc:
        with tc.tile_pool(name="const", bufs=1) as cp, \
             tc.tile_pool(name="glob", bufs=1) as gp, \
             tc.tile_pool(name="dram", bufs=1, space="DRAM") as dram:

            parity = nc.sync.partition_id()
            parity = nc.sync.scalar_reg_alu(ALU.mod, parity, 2)

            ones2 = cp.tile([2, 128], BF16, name="ones2")
            nc.vector.memset(ones2[:], 1.0)
            onescol = cp.tile([128, 1], BF16, name="onescol")
            nc.vector.memset(onescol[:], 1.0)

            # all weights arrive in two packed tensors -> two DMACopies
            # (dozens of small loads would serialize ~30us on the HWDGE)
            wpt = cp.tile([128, wcols], BF16, name="wpt")
            nc.sync.dma_start(wpt[:], wpack_in[:, :])
            fpt = cp.tile([128, fcols], F32, name="fpt")
            nc.sync.dma_start(fpt[:], fpack_in[:, :])
            w8t = cp.tile([128, w8cols], FP8, name="w8t")
            nc.sync.dma_start(w8t[:], w8pack_in[:, :])
            W8 = {}
            for key, pk, pr, c, off in w8lay:
                W8.setdefault(key, []).append(
                    w8t[0:pk, off:off + c].rearrange("p (s m) -> p s m", s=2))
            W = {}
            for key, r, c, off in wlay:
                W.setdefault(key, []).append(wpt[0:r, off:off + c])
            for key, r, c, off in flay:
                W.setdefault(key, []).append(fpt[0:r, off:off + c])
            fw1_tiles = W["fw1"]
            fw2_tiles = W["fw2"]
            fw3_tiles = W["fw3"]
            selB = W["selB"]
            ident = W["ident"][0]
            fbs = {nm: W[nm] for nm in ("fb1", "fb2", "fb3")}

            # persistent double-buffered per-tile structures
            NPIPE = 4
            NCOMP = 6
            comp = []
            for i in range(NCOMP):
                t = gp.tile([128, P], U32, name=f"comp{i}")
                nc.gpsimd.iota(t[:], [[1, P]], base=0, channel_multiplier=0)
                comp.append(t)
            wrapped = []
            for i in range(NPIPE):
                t = gp.tile([128, 8 * K], I16, name=f"wrap{i}")
                nc.vector.memset(t[:], 0)
                wrapped.append(t)
            fcbuf = [gp.tile([128, NQ], BF16, name=f"fcbuf{i}") for i in range(2)]
            ytb = [gp.tile([1, 512], F32, name=f"ytb{i}") for i in range(2)]
            segtop = [gp.tile([128, 64], F32, name=f"segtop{i}")
                      for i in range(8)]
            top24 = [gp.tile([128, 24], F32, name=f"top24{i}")
                     for i in range(NPIPE)]
            idx16 = [gp.tile([128, 24], I16, name=f"idx16{i}")
                     for i in range(NPIPE)]
            idx_dram = [dram.tile([128, K], I16, name=f"idxd{i}")
                        for i in range(NPIPE)]

            # global fp8 point-major gather table: rows D..256 stay zero
            # (host zeros at L1; later layers only overwrite their D columns)
            xsb8 = gp.tile([128, 16, 256], FP8, name="xsb8")
            nc.vector.memset(xsb8[:], 0.0)

            # resident per-layer outputs (feature-major) for the FC head
            xoT = {}
            for li, (_, _, _, DO) in enumerate(LCFG, start=1):
                xoT[li] = [gp.tile([min(128, DO - c0), NQ], BF16,
                                   name=f"xoT{li}_{c0}")
                           for c0 in range(0, DO, 128)]

            ag_in = [[dram.tile([do, NQ // 2], BF16, name=f"agin{li}_{h}")
                      for h in range(2)]
                     for li, (_, _, _, do) in enumerate(LCFG[:3], start=1)]
            ag_out = [[dram.tile([2 * do, NQ // 2], BF16, name=f"agout{li}_{h}")
                       for h in range(2)]
                      for li, (_, _, _, do) in enumerate(LCFG[:3], start=1)]

            nsq2 = None    # [2, P] bf16 hi/lo of -0.5|x|^2 (layers 2-4)

            for li, (D, DP, DH, DO) in enumerate(LCFG, start=1):
                NDC = cdiv(D, 128)     # unpadded contract chunks (dist, u)
                NDCP = DP // 128       # padded contract chunks (gather/h1)
                NHC = cdiv(DH, 128)
                NMC = cdiv(DO, 128)
                h2_bufs = 2 if NMC <= 2 else 1
                h1_bufs = 2
                ps_bufs = 4 if NMC == 1 else 2

                with tc.tile_pool(name=f"l{li}", bufs=1) as lp, \
                     tc.tile_pool(name=f"l{li}w", bufs=2) as wkp, \
                     tc.tile_pool(name=f"l{li}ps", bufs=ps_bufs, space="PSUM") as pdist, \
                     tc.tile_pool(name=f"l{li}h1", bufs=h1_bufs, space="PSUM") as ph1, \
                     tc.tile_pool(name=f"l{li}h2", bufs=h2_bufs, space="PSUM") as ph2:

                    # ---- layer inputs, own-first index space ----
                    # own queries occupy candidate columns 0..NQ; the twin
                    # core's half occupies NQ..P. xq = own features (local,
                    # pre-AllGather); xoth = twin half (post-AllGather).
                    if li == 1:
                        xlhs5 = lp.tile([5, NQ], BF16, name="xlhs1")
                        xro5 = lp.tile([5, NQ], BF16, name="xro1")
                        xrt5 = lp.tile([5, NQ], BF16, name="xrt1")
                        nc.scalar.dma_start(xlhs5[:], xin_in[0:5, :])
                        nc.scalar.dma_start(xro5[:], xin_in[5:10, :])
                        nc.scalar.dma_start(xrt5[:], xin_in[10:15, :])
                        xlhs = [xlhs5[:]]          # features + ones rows
                        xro = [xro5[:]]            # rhs own: features + nsq
                        xrt = [xrt5[:]]            # rhs twin: features + nsq
                        xsb = lp.tile([128, 16, 128], BF16, name="xsb1")
                        nc.scalar.dma_start(
                            xsb[:].rearrange("p r d -> p (r d)"), xsb1_in[:, :])
                    else:
                        DPREV = LCFG[li - 2][3]
                        xq = xoT[li - 1]  # own features, already feature-major
                        xlhs = xq
                        if li not in FP8H1:
                            xsb = lp.tile([128, 16, DP], BF16, name=f"xsb{li}")
                            if DPREV < DP:
                                nc.vector.memset(xsb[:, :, DPREV:DP], 0.0)

                    # pre-AG own-half work: u, xsb own ranks, nsq own quarters
                    usb = []
                    for t in range(8):
                        ups = pdist.tile([128, 512], F32, name="ups", tag="dps")
                        tsl = slice(t * 128, (t + 1) * 128)
                        if li == 1:
                            nc.tensor.matmul(ups[:, :DH], xlhs[0][:, tsl],
                                             W["wdiff1"][0], start=True, stop=True)
                        else:
                            for ci in range(NDC):
                                nc.tensor.matmul(ups[:, :DH], xq[ci][:, tsl],
                                                 W[f"wdiff{li}"][ci],
                                                 start=(ci == 0), stop=False)
                            nc.tensor.matmul(ups[:, :DH], ones2[:],
                                             W[f"ba{li}"][0], start=False, stop=True)
                        ut = lp.tile([128, DH], BF16, name=f"usb{t}")
                        nc.scalar.activation(ut[:], ups[:, :DH], AF.Copy)
                        usb.append(ut)

                    def build_table(src_chunks, half):
                        if li not in FP8H1:
                            for ci, xt in enumerate(src_chunks):
                                eng = nc.scalar if half == 0 else nc.sync
                                eng.dma_start_transpose(
                                    xsb[:, half * 8:half * 8 + 8,
                                        ci * 128:ci * 128 + xt.shape[0]], xt[:])
                            return
                        for ci, xt in enumerate(src_chunks):
                            r = xt.shape[0]
                            for pb in range(8):
                                tp = pdist.tile([128, 512], F32, name="tp",
                                                tag="dps")
                                tv = tp[:].bitcast(BF16)[:, 0:r]
                                nc.tensor.transpose(
                                    tv, xt[:, pb * 128:(pb + 1) * 128],
                                    ident[:r, :r])
                                nc.scalar.activation(
                                    xsb8[:, half * 8 + pb,
                                         ci * 128:ci * 128 + r],
                                    tv, AF.Copy)

                    if li > 1:
                        DPREV = LCFG[li - 2][3]
                        build_table(xq, 0)
                        nsq2 = lp.tile([2, P], BF16, name=f"nsq{li}")
                        nsqlo = lp.tile([1, P], BF16, name=f"nsqlo{li}")
                        sqb = lp.tile([128, NQ], BF16, name=f"sqb{li}")

                        def nsq_quarters(src, base):
                            for nb in range(2):
                                nsqps = pdist.tile([128, 512], F32,
                                                   name="nsqps", tag="dps")
                                for ci, xt in enumerate(src):
                                    r = xt.shape[0]
                                    sl = slice(nb * 512, (nb + 1) * 512)
                                    osl = slice(base + nb * 512,
                                                base + (nb + 1) * 512)
                                    nc.vector.tensor_tensor(
                                        sqb[:r, sl], xt[:, sl], xt[:, sl],
                                        op=ALU.mult)
                                    nc.tensor.matmul(
                                        nsqps[0:1, :], onescol[:r, :],
                                        sqb[:r, sl], start=(ci == 0),
                                        stop=(ci == len(src) - 1))
                                nc.scalar.activation(
                                    nsq2[0:1, osl], nsqps[0:1, :],
                                    AF.Copy, scale=-0.5)
                                nc.vector.scalar_tensor_tensor(
                                    nsqlo[0:1, osl], nsqps[0:1, :],
                                    -0.5, nsq2[0:1, osl],
                                    op0=ALU.mult, op1=ALU.subtract)

                        nsq_quarters(xq, 0)
                        nc.sync.dma_start(nsq2[1:2, 0:NQ], nsqlo[0:1, 0:NQ])

                    # phase A: own-half dist; tiles 0-3 pre-loop (covers
                    # the prior AllGather), tiles 4-7 pipelined in-loop
                    def phase_a(t):
                        tsl = slice(t * 128, (t + 1) * 128)
                        for nb in range(2):
                            dps = pdist.tile([128, 512], F32, name="dpsA", tag="dps")
                            sl = slice(nb * 512, (nb + 1) * 512)
                            if li == 1:
                                nc.tensor.matmul(dps[:], xlhs[0][:, tsl],
                                                 xro[0][:, sl],
                                                 start=True, stop=True)
                            else:
                                for ci in range(NDC):
                                    nc.tensor.matmul(dps[:], xq[ci][:, tsl],
                                                     xq[ci][:, sl],
                                                     start=(ci == 0), stop=False)
                                nc.tensor.matmul(dps[:], ones2[:], nsq2[:, sl],
                                                 start=False, stop=True)
                            nc.scalar.activation(
                                comp[t % NCOMP][:].bitcast(BF16)[:, 1::2][:, sl],
                                dps[:], AF.Copy)
                        cfa = comp[t % NCOMP][:].bitcast(F32)
                        for s in range(4):
                            nc.vector.max(segtop[t][:, s * 8:(s + 1) * 8],
                                          cfa[:, s * 256:(s + 1) * 256])

                    for t in range(4):
                        phase_a(t)

                    if li > 1:
                        DPREV = LCFG[li - 2][3]
                        # post-AG other-half inputs
                        othoff = nc.sync.scalar_reg_alu(ALU.mult, parity, -DPREV)
                        othoff = nc.sync.scalar_reg_alu(ALU.add, othoff, DPREV)
                        xoth = []
                        for c0 in range(0, DPREV, 128):
                            r = min(128, DPREV - c0)
                            rowreg = nc.sync.scalar_reg_alu(ALU.add, othoff, c0)
                            t = lp.tile([r, NQ], BF16, name=f"xoth{li}_{c0}")
                            for h in range(2):
                                nc.sync.dma_start(
                                    t[:, h * 512:(h + 1) * 512],
                                    ag_out[li - 2][h][bass.ds(rowreg, r), :])
                            xoth.append(t)
                        build_table(xoth, 1)
                        nsq_quarters(xoth, NQ)
                        nc.sync.dma_start(nsq2[1:2, NQ:P], nsqlo[0:1, NQ:P])

                    if NMC == 1:
                        maccp = None
                        macc = [lp.tile([DO, NQ], BF16, name=f"macc{li}_0")]
                    else:
                        maccp = [lp.tile([128, 2, NQ], BF16,
                                         name=f"maccp{li}_{mp}")
                                 for mp in range(NMC // 2)]
                        macc = [maccp[m // 2][:, m % 2, :]
                                for m in range(NMC)]

                    # fp8 h1 ring for DoubleRow h2: [pk, 2*NHP, 320]; subtiles
                    # past NHC stay zero (their wb8 rows are zero-padded too,
                    # but 0*NaN from uninitialized SBUF would poison PSUM)
                    PK8 = min(128, DH)
                    NHP = cdiv(max(2, NHC), 2)
                    h1r = [lp.tile([PK8, 2 * NHP, 320], FP8, name=f"h1r{i}")
                           for i in range(4)]
                    if 2 * NHP > NHC:
                        for hb in h1r:
                            nc.vector.memset(hb[:, NHC:2 * NHP, :], 0.0)

                    if li == 4:
                        h1fc = [lp.tile([128, NQ], BF16, name=f"h1fc{m}")
                                for m in range(4)]
                        h2fc = [fcbuf[0], fcbuf[1]]
                        yt_g = [ytb[0][:, :], ytb[1][:, :]]
                        feats = [xoT[1][0], xoT[2][0], xoT[3][0], xoT[3][1],
                                 xoT[4][0], xoT[4][1], xoT[4][2], xoT[4][3]]

                        def emit_fc1(g):
                            gsl = slice(g * 512, (g + 1) * 512)
                            for m in range(4):
                                ps = pdist.tile([128, 512], F32, name="fps",
                                                tag="dps")
                                for ci, ft in enumerate(feats):
                                    nc.tensor.matmul(
                                        ps[:],
                                        fw1_tiles[ci][:, m * 128:(m + 1) * 128],
                                        ft[:, gsl],
                                        start=(ci == 0), stop=(ci == 7))
                                nc.scalar.activation(h1fc[m][:, gsl], ps[:],
                                                     AF.Relu, bias=fbs["fb1"][m])

                        def emit_fc2(g):
                            gsl = slice(g * 512, (g + 1) * 512)
                            for m in range(2):
                                ps = pdist.tile([128, 512], F32, name="fps2",
                                                tag="dps")
                                for ci in range(4):
                                    nc.tensor.matmul(
                                        ps[:],
                                        fw2_tiles[ci][:, m * 128:(m + 1) * 128],
                                        h1fc[ci][:, gsl],
                                        start=(ci == 0), stop=(ci == 3))
                                nc.scalar.activation(h2fc[m][:, gsl], ps[:],
                                                     AF.Relu, bias=fbs["fb2"][m])

                        def emit_fc3(g):
                            gsl = slice(g * 512, (g + 1) * 512)
                            ps = pdist.tile([128, 512], F32, name="fps3",
                                            tag="dps")
                            for ci in range(2):
                                nc.tensor.matmul(ps[0:1, :], fw3_tiles[ci],
                                                 h2fc[ci][:, gsl],
                                                 start=(ci == 0), stop=(ci == 1))
                            nc.scalar.activation(yt_g[g][:, :], ps[0:1, :],
                                                 AF.Sigmoid, bias=fbs["fb3"][0])

                    def emit_xo_half(h):
                        """relu(macc + bb) for column half h -> xoT (+ AG in)."""
                        hsl = slice(h * 512, (h + 1) * 512)
                        for m in range(NMC):
                            mr = min(128, DO - m * 128)
                            nc.scalar.activation(xoT[li][m][:, hsl],
                                                 macc[m][:mr, hsl], AF.Relu,
                                                 bias=W[f"bb{li}"][m])
                            if li < 4:
                                nc.sync.dma_start(
                                    ag_in[li - 1][h][m * 128:m * 128 + mr, :],
                                    xoT[li][m][:, hsl])

                    # ---- main per-tile loop ----
                    for t in range(8):
                        if t == 4:
                            emit_xo_half(0)
                        if t == 5 and li < 4:
                            nc.gpsimd.collective_compute(
                                "AllGather", ALU.bypass, replica_groups=groups,
                                ins=[ag_in[li - 1][0].opt()],
                                outs=[ag_out[li - 1][0].opt()])
                        if li == 4:
                            if t == 5:
                                emit_fc1(0)
                            elif t == 6:
                                emit_fc2(0)
                            elif t == 7:
                                emit_fc3(0)
                        tq = t % NPIPE
                        if t < 4:
                            phase_a(t + 4)
                        tsl = slice(t * 128, (t + 1) * 128)
                        # other-half dist quarters -> bf16 scores written
                        # straight into the composite's odd u16 lanes
                        cb = comp[t % NCOMP]
                        cbv = cb[:].bitcast(BF16)[:, 1::2]
                        for nb in range(2):
                            dps = pdist.tile([128, 512], F32, name="dps", tag="dps")
                            sl = slice(NQ + nb * 512, NQ + (nb + 1) * 512)
                            rsl = slice(nb * 512, (nb + 1) * 512)
                            if li == 1:
                                nc.tensor.matmul(dps[:], xlhs[0][:, tsl],
                                                 xrt[0][:, rsl],
                                                 start=True, stop=True)
                            else:
                                for ci in range(NDC):
                                    nc.tensor.matmul(dps[:], xq[ci][:, tsl],
                                                     xoth[ci][:, rsl],
                                                     start=(ci == 0), stop=False)
                                nc.tensor.matmul(dps[:], ones2[:], nsq2[:, sl],
                                                 start=False, stop=True)
                            nc.scalar.activation(cbv[:, sl], dps[:], AF.Copy)
                        compf = cb[:].bitcast(F32)
                        st = segtop[t]
                        for s in range(4, 8):
                            nc.vector.max(st[:, s * 8:(s + 1) * 8],
                                          compf[:, s * 256:(s + 1) * 256])
                        t24 = top24[tq]
                        for r in range(3):
                            nc.vector.max(t24[:, 8 * r:8 * r + 8], st[:])
                            if r < 2:
                                nc.vector.match_replace(
                                    st[:], t24[:, 8 * r:8 * r + 8], st[:], NEG)
                        nc.vector.tensor_copy(idx16[tq][:],
                                              t24[:].bitcast(I16)[:, 0::2])
                        # wrap indices: dram bounce + replication into the
                        # interpreter block (0) and per-queue TX blocks
                        nc.sync.dma_start(idx_dram[tq][:, :], idx16[tq][:, 0:K])
                        wsrc = idx_dram[tq][:, :].rearrange(
                            "(qh ql) k -> ql qh k", ql=16)
                        for bb in GBLOCKS:
                            nc.sync.dma_start(
                                wrapped[tq][bb * 16:(bb + 1) * 16, :].rearrange(
                                    "ql (qh k) -> ql qh k", k=K), wsrc)
                        # transposed gathers: NGA x (2560/NGA) edges
                        nper = 2560 // NGA
                        vkc = []
                        for g in range(NGA):
                            f8 = li in FP8H1
                            vt = wkp.tile(
                                [128, 2 if f8 else NDCP, nper],
                                FP8 if f8 else BF16,
                                name=f"vt{g}", tag=f"vt{g}")
                            gsrc = xsb8 if f8 else xsb
                            nc.gpsimd.dma_gather(
                                out_ap=vt[:],
                                in_ap=gsrc[:].rearrange("p r d -> p (r d)"),
                                idxs_ap=wrapped[tq][:, g * (nper // 16):
                                                    (g + 1) * (nper // 16)],
                                num_idxs=nper, num_idxs_reg=nper,
                                elem_size=256 if f8 else DP,
                                transpose=True, queue_num=g,
                                sbuf_tokens_per_rank=128,
                                sbuf_free_dim_per_rank=256 if f8 else DP * 2,
                                sbuf_free_dim_pad_per_rank=0,
                                sbuf_byte_offset=0)
                            vkc.append(vt)
                        # edge MLP per qh block (320 edges)
                        qh_per_g = 8 // NGA
                        for qh in range(8):
                            vt = vkc[qh // qh_per_g]
                            off = (qh % qh_per_g) * 320
                            h1sb = h1r[(t * 8 + qh) % 4]
                            for hc in range(NHC):
                                hr = min(128, DH - hc * 128)
                                h1ps = ph1.tile([128, 320], F32, name="h1ps",
                                                tag="h1ps")
                                if li in FP8H1:
                                    nc.tensor.matmul(
                                        h1ps[:hr, :],
                                        W8[f"wbot8_{li}"][0][
                                            :, :, hc * 128:hc * 128 + hr],
                                        vt[:, :, off:off + 320],
                                        start=True, stop=False,
                                        perf_mode=mybir.MatmulPerfMode.DoubleRow)
                                else:
                                    for dc in range(NDCP):
                                        nc.tensor.matmul(
                                            h1ps[:hr, :],
                                            W[f"wbot{li}"][dc][
                                                :, hc * 128:hc * 128 + hr],
                                            vt[:, dc, off:off + 320],
                                            start=(dc == 0), stop=False)
                                nc.tensor.matmul(
                                    h1ps[:hr, :],
                                    usb[t][:, hc * 128:hc * 128 + hr],
                                    selB[qh][:], start=False, stop=True)
                                nc.scalar.activation(h1sb[:hr, hc, :],
                                                     h1ps[:hr, :], AF.Relu)
                            if NMC == 1:
                                h2ps = ph2.tile([128, 320], F32, name="h2ps",
                                                tag="h2_0")
                                for hp in range(NHP):
                                    nc.tensor.matmul(
                                        h2ps[:DO, :],
                                        W8[f"wb8_{li}"][hp][:, :, 0:DO],
                                        h1sb[:, 2 * hp:2 * hp + 2, :],
                                        start=(hp == 0), stop=(hp == NHP - 1),
                                        perf_mode=mybir.MatmulPerfMode.DoubleRow)
                                nc.vector.tensor_reduce(
                                    macc[0][:DO, t * 128 + qh * 16:
                                            t * 128 + qh * 16 + 16],
                                    h2ps[:DO, :].rearrange("p (k ql) -> p ql k",
                                                           k=K),
                                    axis=AX.X, op=ALU.max)
                            else:
                                for mp in range(NMC // 2):
                                    h2pt = ph2.tile([128, 2, 512], F32,
                                                    name="h2pt", tag=f"h2p_{mp}")
                                    for mi in range(2):
                                        m = 2 * mp + mi
                                        for hp in range(NHP):
                                            nc.tensor.matmul(
                                                h2pt[:128, mi, 0:320],
                                                W8[f"wb8_{li}"][hp][:, :,
                                                    m * 128:(m + 1) * 128],
                                                h1sb[:, 2 * hp:2 * hp + 2, :],
                                                start=(hp == 0),
                                                stop=(hp == NHP - 1),
                                                perf_mode=
                                                mybir.MatmulPerfMode.DoubleRow)
                                    c0 = t * 128 + qh * 16
                                    nc.vector.tensor_reduce(
                                        maccp[mp][:, :, c0:c0 + 16],
                                        h2pt[:, :, 0:320].rearrange(
                                            "p pr (k ql) -> p pr ql k", k=K),
                                        axis=AX.X, op=ALU.max)

                    emit_xo_half(1)
                    if li < 4:
                        nc.gpsimd.collective_compute(
                            "AllGather", ALU.bypass, replica_groups=groups,
                            ins=[ag_in[li - 1][1].opt()],
                            outs=[ag_out[li - 1][1].opt()])
                    if li == 4:
                        emit_fc1(1)
                        emit_fc2(1)
                        emit_fc3(1)
                        nc.sync.dma_start(y_out[:, 0:512], yt_g[0][:, :])
                        nc.sync.dma_start(y_out[:, 512:NQ], yt_g[1][:, :])

    nc.compile()
    return nc


def _bf16(a):
    return np.asarray(a, np.float32).astype(ml_dtypes.bfloat16)


def _hilo(row):
    """f32 row -> [2, N] bf16 (hi, residual)."""
    hi = row.astype(ml_dtypes.bfloat16)
    lo = (row - hi.astype(np.float32)).astype(ml_dtypes.bfloat16)
    return np.stack([hi.astype(np.float32), lo.astype(np.float32)]).astype(
        ml_dtypes.bfloat16)


def kernel(**inputs):
    x = np.asarray(inputs["x"], np.float32)          # [8192, 3]
    if "nc" not in _CACHED:
        _CACHED["nc"] = _build()
    nc = _CACHED["nc"]

    full, fullf = {}, {}
    for li, (D, DP, DH, DO) in enumerate(LCFG, start=1):
        wa = np.asarray(inputs[f"w{li}a"], np.float32)
        wtop, wbot = wa[:D], wa[D:]
        ba_hl = _hilo(np.asarray(inputs[f"b{li}a"], np.float32)).astype(
            np.float32)
        if li == 1:
            full["wdiff1"] = _bf16(np.concatenate([wtop - wbot, ba_hl]))
        else:
            full[f"wdiff{li}"] = _bf16(wtop - wbot)
            full[f"ba{li}"] = _bf16(ba_hl)
        if li not in FP8H1:
            wbp = np.zeros((DP, DH), np.float32)
            wbp[:D] = wbot
            full[f"wbot{li}"] = _bf16(wbp)
        fullf[f"bb{li}"] = np.asarray(inputs[f"b{li}b"], np.float32)[:, None]
    # selB[qh][q, k*16+ql] = 1 iff q == qh*16+ql   (stacked row-wise: the
    # packer slices consecutive 128-row chunks from a [8*128, 320] array)
    selB = np.zeros((8 * 128, 320), np.float32)
    for qh in range(8):
        for k in range(K):
            for ql in range(16):
                selB[qh * 128 + qh * 16 + ql, k * 16 + ql] = 1.0
    full["selB"] = _bf16(selB)
    full["ident"] = _bf16(np.eye(128, dtype=np.float32))
    full["fw1"] = _bf16(inputs["fw1"])
    full["fw2"] = _bf16(inputs["fw2"])
    full["fw3"] = _bf16(inputs["fw3"])
    for nm in ("fb1", "fb2", "fb3"):
        fullf[nm] = np.asarray(inputs[nm], np.float32)[:, None]

    wlay, wcols = _wpack_layout()
    w8lay, w8cols = _w8pack_layout()
    flay, fcols = _fpack_layout()
    w8pack = np.zeros((128, w8cols), dtype=ml_dtypes.float8_e4m3)
    for key, pk, pr, c, off in w8lay:
        li8 = int(key.rsplit("_", 1)[1])
        D8, DH8, DO8 = LCFG[li8 - 1][0], LCFG[li8 - 1][2], LCFG[li8 - 1][3]
        if key.startswith("wbot8"):
            wa = np.asarray(inputs[f"w{li8}a"], np.float32)
            wbot_pad = np.zeros((256, DH8), np.float32)
            wbot_pad[:D8] = wa[D8:]
            for s in range(2):
                w8pack[0:128, off + s * DH8:off + (s + 1) * DH8] = \
                    wbot_pad[s * 128:(s + 1) * 128].astype(
                        ml_dtypes.float8_e4m3)
            continue
        wb = np.asarray(inputs[f"w{li8}b"], np.float32)
        for s in range(2):
            r0 = (2 * pr + s) * pk
            blk = np.zeros((pk, DO8), np.float32)
            if r0 < DH8:
                rows = wb[r0:min(r0 + pk, DH8)]
                blk[:rows.shape[0]] = rows
            w8pack[0:pk, off + s * DO8:off + (s + 1) * DO8] = \
                blk.astype(ml_dtypes.float8_e4m3)
    wpack = np.zeros((128, wcols), dtype=ml_dtypes.bfloat16)
    cur = {}
    for key, r, c, off in wlay:
        r0 = cur.get(key, 0)
        wpack[0:r, off:off + c] = full[key][r0:r0 + r, :]
        cur[key] = r0 + r
    fpack = np.zeros((128, fcols), np.float32)
    cur = {}
    for key, r, c, off in flay:
        r0 = cur.get(key, 0)
        fpack[0:r, off:off + c] = fullf[key][r0:r0 + r, :]
        cur[key] = r0 + r

    base = {"wpack": wpack, "w8pack": w8pack, "fpack": fpack}

    in_maps = []
    for c in range(N_CORES):
        cloud, half = c // 2, c % 2
        xc = x[cloud * P:(cloud + 1) * P]
        # own-first reorder: this core's 1024 query points come first
        xown = xc[half * NQ:(half + 1) * NQ]
        xtwin = xc[(1 - half) * NQ:(2 - half) * NQ]
        xr = np.concatenate([xown, xtwin])
        m = dict(base)
        xin = np.zeros((15, NQ), np.float32)
        xin[0:3] = xown.T
        xin[3:5] = 1.0
        xin[5:8] = xown.T
        xin[8:10] = _hilo(-0.5 * (xown * xown).sum(1)).astype(np.float32)
        xin[10:13] = xtwin.T
        xin[13:15] = _hilo(-0.5 * (xtwin * xtwin).sum(1)).astype(np.float32)
        m["xin"] = _bf16(xin)
        xp = np.zeros((P, 128), np.float32)
        xp[:, :3] = xr
        m["xsb1"] = _bf16(
            xp.reshape(16, 128, 128).transpose(1, 0, 2).reshape(128, 16 * 128))
        in_maps.append(m)

    res = run_bass_kernel_spmd(nc, in_maps, core_ids=list(range(N_CORES)))
    out = np.empty((B * P, 1), np.float32)
    for c in range(N_CORES):
        cloud, half = c // 2, c % 2
        out[cloud * P + half * NQ: cloud * P + (half + 1) * NQ, 0] = \
            res.results[c]["y"][0]
    return out


# revision 33
# speedup vs baseline: 1.0052x; 1.0052x over previous
"""DGCNN (4x EdgeConv + FC head) Bass kernel for 8 trn2 NeuronCores — v3.

Sharding: cloud b -> cores {2b, 2b+1}; each core owns 1024 query points
(q0 = (pid % 2) * 1024). Full cloud features exchanged within each pair via
bf16 AllGather after layers 1-3.

v3 changes vs v2 baseline:
- u injected via 128-query selector matmuls (selB[qh]) straight from the
  PSUM-evicted u tile: kills the per-tile DRAM bounce (2 DMAs/tile).
- Single SWDGE queue, one 2560-edge gather per tile (994ns fixed descgen
  cost amortized 4x); index replication shrinks to blocks {0,1}.
- L1 dist/u matmuls fold nsq + bias rows into the 5-row contract (matmul
  cost is free-size only), halving L1 dist PE time.
"""
import numpy as np
import ml_dtypes

import concourse.bass as bass
import concourse.bacc as bacc
import concourse.mybir as mybir
import concourse.tile as tile
from concourse.bass_utils import run_bass_kernel_spmd

B, P, K = 4, 2048, 20
NQ = 1024
N_CORES = 8
NGA = 4            # gathers per tile
FP8H1 = (3, 4)     # layers whose h1 uses the fp8 table + DoubleRow
GBLOCKS = (1, 3, 5, 7)   # wrapped idx replication blocks
NEG = -3.0e38
F32 = mybir.dt.float32
FP8 = mybir.dt.float8e4
BF16 = mybir.dt.bfloat16
U32 = mybir.dt.uint32
U16 = mybir.dt.uint16
I16 = mybir.dt.int16
AF = mybir.ActivationFunctionType
ALU = mybir.AluOpType
AX = mybir.AxisListType

#          D  Dpad  DH   DO
LCFG = [(3, 128, 64, 64),
        (64, 128, 128, 128),
        (128, 128, 256, 256),
        (256, 256, 512, 512)]
FC1_CHUNKS = [64, 128, 128, 128, 128, 128, 128, 128]  # 960 rows

_CACHED = {}


def cdiv(a, b):
    return (a + b - 1) // b


def _wpack_layout():
    """(key, rows, cols, col_off) chunks of the packed bf16 weight tensor."""
    lay, off = [], 0

    def add(key, r, c):
        nonlocal off
        lay.append((key, r, c, off))
        off += c

    for li, (D, DP, DH, DO) in enumerate(LCFG, start=1):
        if li == 1:
            add("wdiff1", 5, DH)  # 3 feature rows + ba hi/lo
        else:
            for c0 in range(0, D, 128):
                add(f"wdiff{li}", min(128, D - c0), DH)
            add(f"ba{li}", 2, DH)
        if li not in FP8H1:
            for c0 in range(0, DP, 128):
                add(f"wbot{li}", 128, DH)
    add("ident", 128, 128)
    for qh in range(8):
        add("selB", 128, 320)
    for r in FC1_CHUNKS:
        add("fw1", r, 512)
    for _ in range(4):
        add("fw2", 128, 256)
    for _ in range(2):
        add("fw3", 128, 1)
    return lay, off


def _w8pack_layout():
    """(key, pk, npair, cols, col_off) chunks of the packed fp8 wb tensor.

    Each chunk holds one DoubleRow pair: [pk, 2, DO] -> 2*DO flat columns.
    Pairs beyond DH are zero-padded (their h1sb subtiles are zeroed too).
    """
    lay, off = [], 0
    for li, (D, DP, DH, DO) in enumerate(LCFG, start=1):
        pk = min(128, DH)
        npair = cdiv(max(2, cdiv(DH, 128)), 2)
        for pr in range(npair):
            lay.append((f"wb8_{li}", pk, pr, 2 * DO, off))
            off += 2 * DO
        lay.append((f"wbot8_{li}", 128, 0, 2 * DH, off))
        off += 2 * DH
    return lay, off


def _fpack_layout():
    """(key, rows, cols, col_off) chunks of the packed f32 bias tensor."""
    lay, off = [], 0

    def add(key, r, c):
        nonlocal off
        lay.append((key, r, c, off))
        off += c

    for li, (_, _, _, DO) in enumerate(LCFG, start=1):
        for c0 in range(0, DO, 128):
            add(f"bb{li}", min(128, DO - c0), 1)
    for c0 in range(0, 512, 128):
        add("fb1", 128, 1)
    for c0 in range(0, 256, 128):
        add("fb2", 128, 1)
    add("fb3", 1, 1)
    return lay, off


def _build():
    nc = bacc.Bacc("TRN2", target_bir_lowering=False, debug=False,
                   num_devices=N_CORES, num_swdge_queues=max(1, NGA),
                   dynamic_dma_scratch_size=16384)

    # ---------------- DRAM params ----------------
    # xin rows: 0-4 lhs (x^T own + two ones), 5-9 rhs-own (x^T own + nsq
    # hi/lo), 10-14 rhs-twin (x^T twin + nsq hi/lo)
    xin_in = nc.declare_dram_parameter("xin", [15, NQ], BF16, isOutput=False)
    xsb1_in = nc.declare_dram_parameter("xsb1", [128, 16 * 128], BF16, isOutput=False)
    wlay, wcols = _wpack_layout()
    w8lay, w8cols = _w8pack_layout()
    flay, fcols = _fpack_layout()
    wpack_in = nc.declare_dram_parameter("wpack", [128, wcols], BF16, isOutput=False)
    w8pack_in = nc.declare_dram_parameter("w8pack", [128, w8cols], FP8,
                                          isOutput=False)
    fpack_in = nc.declare_dram_parameter("fpack", [128, fcols], F32, isOutput=False)
    y_out = nc.declare_dram_parameter("y", [1, NQ], F32, isOutput=True)

    groups = [[2 * b, 2 * b + 1] for b in range(N_CORES // 2)]

    with tile.TileContext(nc) as tc:
        with tc.tile_pool(name="const", bufs=1) as cp, \
             tc.tile_pool(name="glob", bufs=1) as gp, \
             tc.tile_pool(name="dram", bufs=1, space="DRAM") as dram:

            parity = nc.sync.partition_id()
            parity = nc.sync.scalar_reg_alu(ALU.mod, parity, 2)

            ones2 = cp.tile([2, 128], BF16, name="ones2")
            nc.vector.memset(ones2[:], 1.0)
            onescol = cp.tile([128, 1], BF16, name="onescol")
            nc.vector.memset(onescol[:], 1.0)

            # all weights arrive in two packed tensors -> two DMACopies
            # (dozens of small loads would serialize ~30us on the HWDGE)
            wpt = cp.tile([128, wcols], BF16, name="wpt")
            nc.sync.dma_start(wpt[:], wpack_in[:, :])
            fpt = cp.tile([128, fcols], F32, name="fpt")
            nc.sync.dma_start(fpt[:], fpack_in[:, :])
            w8t = cp.tile([128, w8cols], FP8, name="w8t")
            nc.sync.dma_start(w8t[:], w8pack_in[:, :])
            W8 = {}
            for key, pk, pr, c, off in w8lay:
                W8.setdefault(key, []).append(
                    w8t[0:pk, off:off + c].rearrange("p (s m) -> p s m", s=2))
            W = {}
            for key, r, c, off in wlay:
                W.setdefault(key, []).append(wpt[0:r, off:off + c])
            for key, r, c, off in flay:
                W.setdefault(key, []).append(fpt[0:r, off:off + c])
            fw1_tiles = W["fw1"]
            fw2_tiles = W["fw2"]
            fw3_tiles = W["fw3"]
            selB = W["selB"]
            ident = W["ident"][0]
            fbs = {nm: W[nm] for nm in ("fb1", "fb2", "fb3")}

            # persistent double-buffered per-tile structures
            NPIPE = 4
            NCOMP = 6
            comp = []
            for i in range(NCOMP):
                t = gp.tile([128, P], U32, name=f"comp{i}")
                nc.gpsimd.iota(t[:], [[1, P]], base=0, channel_multiplier=0)
                comp.append(t)
            wrapped = []
            for i in range(NPIPE):
                t = gp.tile([128, 8 * K], I16, name=f"wrap{i}")
                nc.vector.memset(t[:], 0)
                wrapped.append(t)
            fcbuf = [gp.tile([128, NQ], BF16, name=f"fcbuf{i}") for i in range(2)]
            ytb = [gp.tile([1, 512], F32, name=f"ytb{i}") for i in range(2)]
            segtop = [gp.tile([128, 64], F32, name=f"segtop{i}")
                      for i in range(8)]
            top24 = [gp.tile([128, 24], F32, name=f"top24{i}")
                     for i in range(NPIPE)]
            idx16 = [gp.tile([128, 24], I16, name=f"idx16{i}")
                     for i in range(NPIPE)]
            idx_dram = [dram.tile([128, K], I16, name=f"idxd{i}")
                        for i in range(NPIPE)]

            # global fp8 point-major gather table: rows D..256 stay zero
            # (host zeros at L1; later layers only overwrite their D columns)
            xsb8 = gp.tile([128, 16, 256], FP8, name="xsb8")
            nc.vector.memset(xsb8[:], 0.0)

            # resident per-layer outputs (feature-major) for the FC head
            xoT = {}
            for li, (_, _, _, DO) in enumerate(LCFG, start=1):
                xoT[li] = [gp.tile([min(128, DO - c0), NQ], BF16,
                                   name=f"xoT{li}_{c0}")
                           for c0 in range(0, DO, 128)]

            ag_in = [[dram.tile([do, NQ // 2], BF16, name=f"agin{li}_{h}")
                      for h in range(2)]
                     for li, (_, _, _, do) in enumerate(LCFG[:3], start=1)]
            ag_out = [[dram.tile([2 * do, NQ // 2], BF16, name=f"agout{li}_{h}")
                       for h in range(2)]
                      for li, (_, _, _, do) in enumerate(LCFG[:3], start=1)]

            nsq2 = None    # [2, P] bf16 hi/lo of -0.5|x|^2 (layers 2-4)

            for li, (D, DP, DH, DO) in enumerate(LCFG, start=1):
                NDC = cdiv(D, 128)     # unpadded contract chunks (dist, u)
                NDCP = DP // 128       # padded contract chunks (gather/h1)
                NHC = cdiv(DH, 128)
                NMC = cdiv(DO, 128)
                h2_bufs = 2 if NMC <= 2 else 1
                h1_bufs = 2
                ps_bufs = 4 if NMC == 1 else 2

                with tc.tile_pool(name=f"l{li}", bufs=1) as lp, \
                     tc.tile_pool(name=f"l{li}w", bufs=2) as wkp, \
                     tc.tile_pool(name=f"l{li}ps", bufs=ps_bufs, space="PSUM") as pdist, \
                     tc.tile_pool(name=f"l{li}h1", bufs=h1_bufs, space="PSUM") as ph1, \
                     tc.tile_pool(name=f"l{li}h2", bufs=h2_bufs, space="PSUM") as ph2:

                    # ---- layer inputs, own-first index space ----
                    # own queries occupy candidate columns 0..NQ; the twin
                    # core's half occupies NQ..P. xq = own features (local,
                    # pre-AllGather); xoth = twin half (post-AllGather).
                    if li == 1:
                        xlhs5 = lp.tile([5, NQ], BF16, name="xlhs1")
                        xro5 = lp.tile([5, NQ], BF16, name="xro1")
                        xrt5 = lp.tile([5, NQ], BF16, name="xrt1")
                        nc.scalar.dma_start(xlhs5[:], xin_in[0:5, :])
                        nc.scalar.dma_start(xro5[:], xin_in[5:10, :])
                        nc.scalar.dma_start(xrt5[:], xin_in[10:15, :])
                        xlhs = [xlhs5[:]]          # features + ones rows
                        xro = [xro5[:]]            # rhs own: features + nsq
                        xrt = [xrt5[:]]            # rhs twin: features + nsq
                        xsb = lp.tile([128, 16, 128], BF16, name="xsb1")
                        nc.scalar.dma_start(
                            xsb[:].rearrange("p r d -> p (r d)"), xsb1_in[:, :])
                    else:
                        DPREV = LCFG[li - 2][3]
                        xq = xoT[li - 1]  # own features, already feature-major
                        xlhs = xq
                        if li not in FP8H1:
                            xsb = lp.tile([128, 16, DP], BF16, name=f"xsb{li}")
                            if DPREV < DP:
                                nc.vector.memset(xsb[:, :, DPREV:DP], 0.0)

                    # pre-AG own-half work: u, xsb own ranks, nsq own quarters
                    usb = []
                    for t in range(8):
                        ups = pdist.tile([128, 512], F32, name="ups", tag="dps")
                        tsl = slice(t * 128, (t + 1) * 128)
                        if li == 1:
                            nc.tensor.matmul(ups[:, :DH], xlhs[0][:, tsl],
                                             W["wdiff1"][0], start=True, stop=True)
                        else:
                            for ci in range(NDC):
                                nc.tensor.matmul(ups[:, :DH], xq[ci][:, tsl],
                                                 W[f"wdiff{li}"][ci],
                                                 start=(ci == 0), stop=False)
                            nc.tensor.matmul(ups[:, :DH], ones2[:],
                                             W[f"ba{li}"][0], start=False, stop=True)
                        ut = lp.tile([128, DH], BF16, name=f"usb{t}")
                        nc.scalar.activation(ut[:], ups[:, :DH], AF.Copy)
                        usb.append(ut)

                    def build_table(src_chunks, half):
                        if li not in FP8H1:
                            for ci, xt in enumerate(src_chunks):
                                eng = nc.scalar if half == 0 else nc.sync
                                eng.dma_start_transpose(
                                    xsb[:, half * 8:half * 8 + 8,
                                        ci * 128:ci * 128 + xt.shape[0]], xt[:])
                            return
                        for ci, xt in enumerate(src_chunks):
                            r = xt.shape[0]
                            for pb in range(8):
                                tp = pdist.tile([128, 512], F32, name="tp",
                                                tag="dps")
                                tv = tp[:].bitcast(BF16)[:, 0:r]
                                nc.tensor.transpose(
                                    tv, xt[:, pb * 128:(pb + 1) * 128],
                                    ident[:r, :r])
                                nc.scalar.activation(
                                    xsb8[:, half * 8 + pb,
                                         ci * 128:ci * 128 + r],
                                    tv, AF.Copy)

                    if li > 1:
                        DPREV = LCFG[li - 2][3]
                        build_table(xq, 0)
                        nsq2 = lp.tile([2, P], BF16, name=f"nsq{li}")
                        nsqlo = lp.tile([1, P], BF16, name=f"nsqlo{li}")
                        sqb = lp.tile([128, NQ], BF16, name=f"sqb{li}")

                        def nsq_quarters(src, base):
                            for nb in range(2):
                                nsqps = pdist.tile([128, 512], F32,
                                                   name="nsqps", tag="dps")
                                for ci, xt in enumerate(src):
                                    r = xt.shape[0]
                                    sl = slice(nb * 512, (nb + 1) * 512)
                                    osl = slice(base + nb * 512,
                                                base + (nb + 1) * 512)
                                    nc.vector.tensor_tensor(
                                        sqb[:r, sl], xt[:, sl], xt[:, sl],
                                        op=ALU.mult)
                                    nc.tensor.matmul(
                                        nsqps[0:1, :], onescol[:r, :],
                                        sqb[:r, sl], start=(ci == 0),
                                        stop=(ci == len(src) - 1))
                                nc.scalar.activation(
                                    nsq2[0:1, osl], nsqps[0:1, :],
                                    AF.Copy, scale=-0.5)
                                nc.vector.scalar_tensor_tensor(
                                    nsqlo[0:1, osl], nsqps[0:1, :],
                                    -0.5, nsq2[0:1, osl],
                                    op0=ALU.mult, op1=ALU.subtract)

                        nsq_quarters(xq, 0)
                        nc.sync.dma_start(nsq2[1:2, 0:NQ], nsqlo[0:1, 0:NQ])

                    # phase A: own-half dist; tiles 0-3 pre-loop (covers
                    # the prior AllGather), tiles 4-7 pipelined in-loop
                    def phase_a(t):
                        tsl = slice(t * 128, (t + 1) * 128)
                        for nb in range(2):
                            dps = pdist.tile([128, 512], F32, name="dpsA", tag="dps")
                            sl = slice(nb * 512, (nb + 1) * 512)
                            if li == 1:
                                nc.tensor.matmul(dps[:], xlhs[0][:, tsl],
                                                 xro[0][:, sl],
                                                 start=True, stop=True)
                            else:
                                for ci in range(NDC):
                                    nc.tensor.matmul(dps[:], xq[ci][:, tsl],
                                                     xq[ci][:, sl],
                                                     start=(ci == 0), stop=False)
                                nc.tensor.matmul(dps[:], ones2[:], nsq2[:, sl],
                                                 start=False, stop=True)
                            nc.scalar.activation(
                                comp[t % NCOMP][:].bitcast(BF16)[:, 1::2][:, sl],
                                dps[:], AF.Copy)
                        cfa = comp[t % NCOMP][:].bitcast(F32)
                        for s in range(4):
                            nc.vector.max(segtop[t][:, s * 8:(s + 1) * 8],
                                          cfa[:, s * 256:(s + 1) * 256])

                    for t in range(4):
                        phase_a(t)

                    if li > 1:
                        DPREV = LCFG[li - 2][3]
                        # post-AG other-half inputs
                        othoff = nc.sync.scalar_reg_alu(ALU.mult, parity, -DPREV)
                        othoff = nc.sync.scalar_reg_alu(ALU.add, othoff, DPREV)
                        xoth = []
                        for c0 in range(0, DPREV, 128):
                            r = min(128, DPREV - c0)
                            rowreg = nc.sync.scalar_reg_alu(ALU.add, othoff, c0)
                            t = lp.tile([r, NQ], BF16, name=f"xoth{li}_{c0}")
                            for h in range(2):
                                nc.sync.dma_start(
                                    t[:, h * 512:(h + 1) * 512],
                                    ag_out[li - 2][h][bass.ds(rowreg, r), :])
                            xoth.append(t)
                        build_table(xoth, 1)
                        nsq_quarters(xoth, NQ)
                        nc.sync.dma_start(nsq2[1:2, NQ:P], nsqlo[0:1, NQ:P])

                    if NMC == 1:
                        maccp = None
                        macc = [lp.tile([DO, NQ], BF16, name=f"macc{li}_0")]
                    else:
                        maccp = [lp.tile([128, 2, NQ], BF16,
                                         name=f"maccp{li}_{mp}")
                                 for mp in range(NMC // 2)]
                        macc = [maccp[m // 2][:, m % 2, :]
                                for m in range(NMC)]

                    # fp8 h1 ring for DoubleRow h2: [pk, 2*NHP, 320]; subtiles
                    # past NHC stay zero (their wb8 rows are zero-padded too,
                    # but 0*NaN from uninitialized SBUF would poison PSUM)
                    PK8 = min(128, DH)
                    NHP = cdiv(max(2, NHC), 2)
                    h1r = [lp.tile([PK8, 2 * NHP, 320], FP8, name=f"h1r{i}")
                           for i in range(4)]
                    if 2 * NHP > NHC:
                        for hb in h1r:
                            nc.vector.memset(hb[:, NHC:2 * NHP, :], 0.0)

                    if li == 4:
                        h1fc = [lp.tile([128, NQ], BF16, name=f"h1fc{m}")
                                for m in range(4)]
                        h2fc = [fcbuf[0], fcbuf[1]]
                        yt_g = [ytb[0][:, :], ytb[1][:, :]]
                        feats = [xoT[1][0], xoT[2][0], xoT[3][0], xoT[3][1],
                                 xoT[4][0], xoT[4][1], xoT[4][2], xoT[4][3]]

                        def emit_fc1(g):
                            gsl = slice(g * 512, (g + 1) * 512)
                            for m in range(4):
                                ps = pdist.tile([128, 512], F32, name="fps",
                                                tag="dps")
                                for ci, ft in enumerate(feats):
                                    nc.tensor.matmul(
                                        ps[:],
                                        fw1_tiles[ci][:, m * 128:(m + 1) * 128],
                                        ft[:, gsl],
                                        start=(ci == 0), stop=(ci == 7))
                                nc.scalar.activation(h1fc[m][:, gsl], ps[:],
                                                     AF.Relu, bias=fbs["fb1"][m])

                        def emit_fc2(g):
                            gsl = slice(g * 512, (g + 1) * 512)
                            for m in range(2):
                                ps = pdist.tile([128, 512], F32, name="fps2",
                                                tag="dps")
                                for ci in range(4):
                                    nc.tensor.matmul(
                                        ps[:],
                                        fw2_tiles[ci][:, m * 128:(m + 1) * 128],
                                        h1fc[ci][:, gsl],
                                        start=(ci == 0), stop=(ci == 3))
                                nc.scalar.activation(h2fc[m][:, gsl], ps[:],
                                                     AF.Relu, bias=fbs["fb2"][m])

                        def emit_fc3(g):
                            gsl = slice(g * 512, (g + 1) * 512)
                            ps = pdist.tile([128, 512], F32, name="fps3",
                                            tag="dps")
                            for ci in range(2):
                                nc.tensor.matmul(ps[0:1, :], fw3_tiles[ci],
                                                 h2fc[ci][:, gsl],
                                                 start=(ci == 0), stop=(ci == 1))
                            nc.scalar.activation(yt_g[g][:, :], ps[0:1, :],
                                                 AF.Sigmoid, bias=fbs["fb3"][0])

                    def emit_xo_half(h):
                        """relu(macc + bb) for column half h -> xoT (+ AG in)."""
                        hsl = slice(h * 512, (h + 1) * 512)
                        for m in range(NMC):
                            mr = min(128, DO - m * 128)
                            nc.scalar.activation(xoT[li][m][:, hsl],
                                                 macc[m][:mr, hsl], AF.Relu,
                                                 bias=W[f"bb{li}"][m])
                            if li < 4:
                                nc.sync.dma_start(
                                    ag_in[li - 1][h][m * 128:m * 128 + mr, :],
                                    xoT[li][m][:, hsl])

                    # ---- main per-tile loop ----
                    for t in range(8):
                        if t == 4:
                            emit_xo_half(0)
                        if t == 5 and li < 4:
                            nc.gpsimd.collective_compute(
                                "AllGather", ALU.bypass, replica_groups=groups,
                                ins=[ag_in[li - 1][0].opt()],
                                outs=[ag_out[li - 1][0].opt()])
                        if t == 7 and li == 4:
                            emit_fc1(0)
                        tq = t % NPIPE
                        if t < 4:
                            phase_a(t + 4)
                        tsl = slice(t * 128, (t + 1) * 128)
                        # other-half dist quarters -> bf16 scores written
                        # straight into the composite's odd u16 lanes
                        cb = comp[t % NCOMP]
                        cbv = cb[:].bitcast(BF16)[:, 1::2]
                        for nb in range(2):
                            dps = pdist.tile([128, 512], F32, name="dps", tag="dps")
                            sl = slice(NQ + nb * 512, NQ + (nb + 1) * 512)
                            rsl = slice(nb * 512, (nb + 1) * 512)
                            if li == 1:
                                nc.tensor.matmul(dps[:], xlhs[0][:, tsl],
                                                 xrt[0][:, rsl],
                                                 start=True, stop=True)
                            else:
                                for ci in range(NDC):
                                    nc.tensor.matmul(dps[:], xq[ci][:, tsl],
                                                     xoth[ci][:, rsl],
                                                     start=(ci == 0), stop=False)
                                nc.tensor.matmul(dps[:], ones2[:], nsq2[:, sl],
                                                 start=False, stop=True)
                            nc.scalar.activation(cbv[:, sl], dps[:], AF.Copy)
                        compf = cb[:].bitcast(F32)
                        st = segtop[t]
                        for s in range(4, 8):
                            nc.vector.max(st[:, s * 8:(s + 1) * 8],
                                          compf[:, s * 256:(s + 1) * 256])
                        t24 = top24[tq]
                        for r in range(3):
                            nc.vector.max(t24[:, 8 * r:8 * r + 8], st[:])
                            if r < 2:
                                nc.vector.match_replace(
                                    st[:], t24[:, 8 * r:8 * r + 8], st[:], NEG)
                        nc.vector.tensor_copy(idx16[tq][:],
                                              t24[:].bitcast(I16)[:, 0::2])
                        # wrap indices: dram bounce + replication into the
                        # interpreter block (0) and per-queue TX blocks
                        nc.sync.dma_start(idx_dram[tq][:, :], idx16[tq][:, 0:K])
                        wsrc = idx_dram[tq][:, :].rearrange(
                            "(qh ql) k -> ql qh k", ql=16)
                        for bb in GBLOCKS:
                            nc.sync.dma_start(
                                wrapped[tq][bb * 16:(bb + 1) * 16, :].rearrange(
                                    "ql (qh k) -> ql qh k", k=K), wsrc)
                        # transposed gathers: NGA x (2560/NGA) edges
                        nper = 2560 // NGA
                        vkc = []
                        for g in range(NGA):
                            f8 = li in FP8H1
                            vt = wkp.tile(
                                [128, 2 if f8 else NDCP, nper],
                                FP8 if f8 else BF16,
                                name=f"vt{g}", tag=f"vt{g}")
                            gsrc = xsb8 if f8 else xsb
                            nc.gpsimd.dma_gather(
                                out_ap=vt[:],
                                in_ap=gsrc[:].rearrange("p r d -> p (r d)"),
                                idxs_ap=wrapped[tq][:, g * (nper // 16):
                                                    (g + 1) * (nper // 16)],
                                num_idxs=nper, num_idxs_reg=nper,
                                elem_size=256 if f8 else DP,
                                transpose=True, queue_num=g,
                                sbuf_tokens_per_rank=128,
                                sbuf_free_dim_per_rank=256 if f8 else DP * 2,
                                sbuf_free_dim_pad_per_rank=0,
                                sbuf_byte_offset=0)
                            vkc.append(vt)
                        # edge MLP per qh block (320 edges)
                        qh_per_g = 8 // NGA
                        for qh in range(8):
                            vt = vkc[qh // qh_per_g]
                            off = (qh % qh_per_g) * 320
                            h1sb = h1r[(t * 8 + qh) % 4]
                            for hc in range(NHC):
                                hr = min(128, DH - hc * 128)
                                h1ps = ph1.tile([128, 320], F32, name="h1ps",
                                                tag="h1ps")
                                if li in FP8H1:
                                    nc.tensor.matmul(
                                        h1ps[:hr, :],
                                        W8[f"wbot8_{li}"][0][
                                            :, :, hc * 128:hc * 128 + hr],
                                        vt[:, :, off:off + 320],
                                        start=True, stop=False,
                                        perf_mode=mybir.MatmulPerfMode.DoubleRow)
                                else:
                                    for dc in range(NDCP):
                                        nc.tensor.matmul(
                                            h1ps[:hr, :],
                                            W[f"wbot{li}"][dc][
                                                :, hc * 128:hc * 128 + hr],
                                            vt[:, dc, off:off + 320],
                                            start=(dc == 0), stop=False)
                                nc.tensor.matmul(
                                    h1ps[:hr, :],
                                    usb[t][:, hc * 128:hc * 128 + hr],
                                    selB[qh][:], start=False, stop=True)
                                nc.scalar.activation(h1sb[:hr, hc, :],
                                                     h1ps[:hr, :], AF.Relu)
                            if NMC == 1:
                                h2ps = ph2.tile([128, 320], F32, name="h2ps",
                                                tag="h2_0")
                                for hp in range(NHP):
                                    nc.tensor.matmul(
                                        h2ps[:DO, :],
                                        W8[f"wb8_{li}"][hp][:, :, 0:DO],
                                        h1sb[:, 2 * hp:2 * hp + 2, :],
                                        start=(hp == 0), stop=(hp == NHP - 1),
                                        perf_mode=mybir.MatmulPerfMode.DoubleRow)
                                nc.vector.tensor_reduce(
                                    macc[0][:DO, t * 128 + qh * 16:
                                            t * 128 + qh * 16 + 16],
                                    h2ps[:DO, :].rearrange("p (k ql) -> p ql k",
                                                           k=K),
                                    axis=AX.X, op=ALU.max)
                            else:
                                for mp in range(NMC // 2):
                                    h2pt = ph2.tile([128, 2, 512], F32,
                                                    name="h2pt", tag=f"h2p_{mp}")
                                    for mi in range(2):
                                        m = 2 * mp + mi
                                        for hp in range(NHP):
                                            nc.tensor.matmul(
                                                h2pt[:128, mi, 0:320],
                                                W8[f"wb8_{li}"][hp][:, :,
                                                    m * 128:(m + 1) * 128],
                                                h1sb[:, 2 * hp:2 * hp + 2, :],
                                                start=(hp == 0),
                                                stop=(hp == NHP - 1),
                                                perf_mode=
                                                mybir.MatmulPerfMode.DoubleRow)
                                    c0 = t * 128 + qh * 16
                                    nc.vector.tensor_reduce(
                                        maccp[mp][:, :, c0:c0 + 16],
                                        h2pt[:, :, 0:320].rearrange(
                                            "p pr (k ql) -> p pr ql k", k=K),
                                        axis=AX.X, op=ALU.max)

                    emit_xo_half(1)
                    if li < 4:
                        nc.gpsimd.collective_compute(
                            "AllGather", ALU.bypass, replica_groups=groups,
                            ins=[ag_in[li - 1][1].opt()],
                            outs=[ag_out[li - 1][1].opt()])
                    if li == 4:
                        emit_fc2(0)
                        emit_fc1(1)
                        emit_fc3(0)
                        emit_fc2(1)
                        emit_fc3(1)
                        nc.sync.dma_start(y_out[:, 0:512], yt_g[0][:, :])
                        nc.sync.dma_start(y_out[:, 512:NQ], yt_g[1][:, :])

    nc.compile()
    return nc


def _bf16(a):
    return np.asarray(a, np.float32).astype(ml_dtypes.bfloat16)


def _hilo(row):
    """f32 row -> [2, N] bf16 (hi, residual)."""
    hi = row.astype(ml_dtypes.bfloat16)
    lo = (row - hi.astype(np.float32)).astype(ml_dtypes.bfloat16)
    return np.stack([hi.astype(np.float32), lo.astype(np.float32)]).astype(
        ml_dtypes.bfloat16)


def kernel(**inputs):
    x = np.asarray(inputs["x"], np.float32)          # [8192, 3]
    if "nc" not in _CACHED:
        _CACHED["nc"] = _build()
    nc = _CACHED["nc"]

    full, fullf = {}, {}
    for li, (D, DP, DH, DO) in enumerate(LCFG, start=1):
        wa = np.asarray(inputs[f"w{li}a"], np.float32)
        wtop, wbot = wa[:D], wa[D:]
        ba_hl = _hilo(np.asarray(inputs[f"b{li}a"], np.float32)).astype(
            np.float32)
        if li == 1:
            full["wdiff1"] = _bf16(np.concatenate([wtop - wbot, ba_hl]))
        else:
            full[f"wdiff{li}"] = _bf16(wtop - wbot)
            full[f"ba{li}"] = _bf16(ba_hl)
        if li not in FP8H1:
            wbp = np.zeros((DP, DH), np.float32)
            wbp[:D] = wbot
            full[f"wbot{li}"] = _bf16(wbp)
        fullf[f"bb{li}"] = np.asarray(inputs[f"b{li}b"], np.float32)[:, None]
    # selB[qh][q, k*16+ql] = 1 iff q == qh*16+ql   (stacked row-wise: the
    # packer slices consecutive 128-row chunks from a [8*128, 320] array)
    selB = np.zeros((8 * 128, 320), np.float32)
    for qh in range(8):
        for k in range(K):
            for ql in range(16):
                selB[qh * 128 + qh * 16 + ql, k * 16 + ql] = 1.0
    full["selB"] = _bf16(selB)
    full["ident"] = _bf16(np.eye(128, dtype=np.float32))
    full["fw1"] = _bf16(inputs["fw1"])
    full["fw2"] = _bf16(inputs["fw2"])
    full["fw3"] = _bf16(inputs["fw3"])
    for nm in ("fb1", "fb2", "fb3"):
        fullf[nm] = np.asarray(inputs[nm], np.float32)[:, None]

    wlay, wcols = _wpack_layout()
    w8lay, w8cols = _w8pack_layout()
    flay, fcols = _fpack_layout()
    w8pack = np.zeros((128, w8cols), dtype=ml_dtypes.float8_e4m3)
    for key, pk, pr, c, off in w8lay:
        li8 = int(key.rsplit("_", 1)[1])
        D8, DH8, DO8 = LCFG[li8 - 1][0], LCFG[li8 - 1][2], LCFG[li8 - 1][3]
        if key.startswith("wbot8"):
            wa = np.asarray(inputs[f"w{li8}a"], np.float32)
            wbot_pad = np.zeros((256, DH8), np.float32)
            wbot_pad[:D8] = wa[D8:]
            for s in range(2):
                w8pack[0:128, off + s * DH8:off + (s + 1) * DH8] = \
                    wbot_pad[s * 128:(s + 1) * 128].astype(
                        ml_dtypes.float8_e4m3)
            continue
        wb = np.asarray(inputs[f"w{li8}b"], np.float32)
        for s in range(2):
            r0 = (2 * pr + s) * pk
            blk = np.zeros((pk, DO8), np.float32)
            if r0 < DH8:
                rows = wb[r0:min(r0 + pk, DH8)]
                blk[:rows.shape[0]] = rows
            w8pack[0:pk, off + s * DO8:off + (s + 1) * DO8] = \
                blk.astype(ml_dtypes.float8_e4m3)
    wpack = np.zeros((128, wcols), dtype=ml_dtypes.bfloat16)
    cur = {}
    for key, r, c, off in wlay:
        r0 = cur.get(key, 0)
        wpack[0:r, off:off + c] = full[key][r0:r0 + r, :]
        cur[key] = r0 + r
    fpack = np.zeros((128, fcols), np.float32)
    cur = {}
    for key, r, c, off in flay:
        r0 = cur.get(key, 0)
        fpack[0:r, off:off + c] = fullf[key][r0:r0 + r, :]
        cur[key] = r0 + r

    base = {"wpack": wpack, "w8pack": w8pack, "fpack": fpack}

    in_maps = []
    for c in range(N_CORES):
        cloud, half = c // 2, c % 2
        xc = x[cloud * P:(cloud + 1) * P]
        # own-first reorder: this core's 1024 query points come first
        xown = xc[half * NQ:(half + 1) * NQ]
        xtwin = xc[(1 - half) * NQ:(2 - half) * NQ]
        xr = np.concatenate([xown, xtwin])
        m = dict(base)
        xin = np.zeros((15, NQ), np.float32)
        xin[0:3] = xown.T
        xin[3:5] = 1.0
        xin[5:8] = xown.T
        xin[8:10] = _hilo(-0.5 * (xown * xown).sum(1)).astype(np.float32)
        xin[10:13] = xtwin.T
        xin[13:15] = _hilo(-0.5 * (xtwin * xtwin).sum(1)).astype(np.float32)
        m["xin"] = _bf16(xin)
        xp = np.zeros((P, 128), np.float32)
        xp[:, :3] = xr
        m["xsb1"] = _bf16(
            xp.reshape(16, 128, 128).transpose(1, 0, 2).reshape(128, 16 * 128))
        in_maps.append(m)

    res = run_bass_kernel_spmd(nc, in_maps, core_ids=list(range(N_CORES)))
    out = np.empty((B * P, 1), np.float32)
    for c in range(N_CORES):
        cloud, half = c // 2, c % 2
        out[cloud * P + half * NQ: cloud * P + (half + 1) * NQ, 0] = \
            res.results[c]["y"][0]
    return out
